# revision 21
# baseline (speedup 1.0000x reference)
"""Trainium2 Bass kernel for the branched cross-attention processor.

Problem (full shapes):
  hidden_states [4, 4096, 1280], encoder_hidden_states [4, 77, 2048],
  id_embedding [2, 32, 2048], Wq/Wout [1280,1280], Wk/Wv/Wid_k/Wid_v
  [2048,1280], bout [1280].  20 heads, dh=64.  Output [4, 4096, 1280].

Sharding: data-parallel over (batch, seq-half): core c handles batch c//2,
query rows (c%2)*2048 : (c%2+1)*2048.  K/V (109 keys) are computed
per-core for its batch.  No collectives.

v2 schedule (PE-roofline oriented, ~255us of fp16 PE work per core):
  B. Q projection, i-outer: for c-chunk, for j-group: accumulate over all
     10 input tiles into <=4 PSUM banks.  Starts as soon as hsT tile 0 +
     the j-group's weights land (weights packed for big-burst DMA).
  C. KV projection (both [Wk|Wv] and [Wid_k|Wid_v] passes); the 21MB
     weight stream overlaps phase B's 85us of PE work.  kT via DMA
     transpose; v stays key-major.
  A. Attention chunk c0 (pairs of heads; scores -> exp -> PV + ones-matmul
     denominator -> reciprocal -> normalize), then chunks c1..c3 with the
     O-projection tiles of chunk c-1 interleaved 2 pairs : 1 tile so the
     ACT exp stream hides under O matmuls and the PE never stalls.
  O. Remaining O tiles of chunk c3 + bias + output DMA.

Engines: PE matmuls; ACT exp + PSUM->SBUF fp16 copies; DVE reciprocal +
bias add; Pool (gpsimd) the normalize multiply.
"""

import os
import sys
import types

import numpy as np

# ---------------------------------------------------------------------------
# problem constants (hardcoded; kernel.py must be self-contained)
# ---------------------------------------------------------------------------
B = 4
S = 4096
H = 1280
C = 2048
TE = 77          # encoder tokens
TI = 32          # id tokens
HEADS = 20
DH = 64          # head dim
P = 128
L = 109          # TE + TI
LP = 128         # padded key count: [0:77]=ehs, [77:96]=gap, [96:128]=id
GAP0, GAP1 = TE, P - TI   # 77, 96
SC = 2048        # seq rows per core
NJ = H // P      # 10
NI = C // P      # 16
NCH = SC // 512  # 4 sq-chunks of 512
NT = SC // P     # 16 sq-tiles of 128
SCALE = 1.0 / 8.0
NCORES = 8
MCHUNKS = [(0, 512), (512, 512), (1024, 256)]
JGS = [(0, 4), (4, 3), (7, 3)]   # j-groups for the i-outer Q passes

_NC_CACHE = {}


def _ensure_axon_hooks():
    """The image's antenv lacks axon_hooks; synthesize it so NTFF profiling
    (trace=True) works when test.py asks for it.  Harmless if unused."""
    if "antenv.axon_hooks" in sys.modules:
        return
    try:
        import antenv
        from trn_agent_boot.trn_boot import _ntff_profile_via_ctypes

        hook = _ntff_profile_via_ctypes("/opt/axon/libaxon_pjrt.so")
        m = types.ModuleType("antenv.axon_hooks")
        m.get_axon_ntff_profile_hook = lambda: hook
        m.set_axon_ntff_profile_hook = lambda h: None
        sys.modules["antenv.axon_hooks"] = m
        antenv.axon_hooks = m
    except Exception:
        pass


def build_nc():
    """Build + compile the per-core Bass program (SPMD: same NEFF, 8 cores)."""
    if "nc" in _NC_CACHE:
        return _NC_CACHE["nc"]

    import concourse.bass as bass
    import concourse.tile as tile
    from concourse import bacc, mybir
    from concourse.bass import ts

    F32 = mybir.dt.float32
    R = mybir.dt.float16      # matmul operand dtype (1 cyc/row)
    EXP = mybir.ActivationFunctionType.Exp

    nc = bacc.Bacc("TRN2", target_bir_lowering=False, debug=False, num_devices=NCORES)

    hsT = nc.dram_tensor("hsT", [H, SC], R, kind="ExternalInput").ap()
    # xkvTp: [128, 16, 128]  (partition-major packed transposed kv inputs)
    xkvTp = nc.dram_tensor("xkvTp", [P, NI, LP], R, kind="ExternalInput").ap()
    # wq packed per j-group: [128, 10, jw*128]
    wqg = [nc.dram_tensor(f"wqg{g}", [P, NJ, jw * P], R, kind="ExternalInput").ap()
           for g, (j0, jw) in enumerate(JGS)]
    # this core's HALF of the kv weights (even cores: k~ cols, odd: v cols),
    # [proj, nn(5x256 outcols), half, 128, 8, 256]; the halves are swapped
    # back by an all-gather of the projected [128, 1280] pieces.
    wkvh = nc.dram_tensor("wkvh", [2, 5, 2, P, 8, 256], R, kind="ExternalInput").ap()
    woutT = nc.dram_tensor("woutT", [H, H], R, kind="ExternalInput").ap()
    boutb = nc.dram_tensor("boutb", [P, H], F32, kind="ExternalInput").ap()
    out = nc.dram_tensor("out", [SC, H], F32, kind="ExternalOutput").ap()

    with tile.TileContext(nc) as tc:
        with tc.tile_pool(name="pers", bufs=1) as pers:
            # ---- persistent constants / small arrays --------------------------
            ones_mat = pers.tile([P, P], R, tag="ones_mat")
            nc.vector.memset(ones_mat[:, :], 1.0)
            bias_col = pers.tile([P, 1], F32, tag="bias_col")
            # engine ops need 32-aligned start partitions: write the gap
            # as [64:96] then restore [64:77]; later writes overwrite cleanly.
            nc.vector.memset(bias_col[:, :], 0.0)
            nc.vector.memset(bias_col[64:GAP1, :], -1e30)
            nc.vector.memset(bias_col[64:GAP0, :], 0.0)
            kT_sb = [pers.tile([P, LP], R, tag=f"kT{j}", name=f"kT{j}") for j in range(NJ)]
            v_sb = pers.tile([LP, HEADS * DH], R, tag="v")
            xkvT_sb = pers.tile([P, NI, LP], R, tag="xkvT")
            wq_sb = [pers.tile([P, NJ, jw * P], R, tag=f"wqg{g}", name=f"wqg{g}")
                     for g, (j0, jw) in enumerate(JGS)]

            # right-side stack: tensors that live until the end
            attnp_cm = tc.tile_pool(name="attnp", bufs=1, side="right")
            attnp = attnp_cm.__enter__()
            attnT_sb = [attnp.tile([P, SC], R, tag=f"attnT{d}", name=f"attnT{d}") for d in range(NJ)]

            qTp_cm = tc.tile_pool(name="qTp", bufs=1)
            qTp = qTp_cm.__enter__()
            qT_sb = [qTp.tile([P, SC], R, tag=f"qT{j}", name=f"qT{j}") for j in range(NJ)]

            kTMP = pers.tile([P, H], R, tag="kTMP")
            # my projected half: even cores k~ [128 keys, 1280], odd cores v
            mykv = pers.tile([P, H], R, tag="mykv")

            # ---- phase B+C: q projection (i-outer passes) with the kv
            # projection chunks interleaved between passes so the kv weight
            # stream spreads over the whole ~110us window -------------------
            with (
                tc.tile_pool(name="wkvp", bufs=3) as wkvp,
                tc.tile_pool(name="phq", bufs=1) as phq,
                tc.tile_pool(name="psq", bufs=6, space="PSUM") as psq,
                tc.tile_pool(name="pskv", bufs=2, space="PSUM") as pskv,
                tc.tile_pool(name="dram", bufs=1, space="DRAM") as dram,
            ):
                # DMA order = priority order: xkv (kv chunk 0), then the
                # first j-group's weights interleaved with the hsT stream
                # (pass (c0, g0) consumes hsT tile-by-tile), then the rest.
                hsT_sb = [phq.tile([P, SC], R, tag=f"hsT{i}", name=f"hsT{i}") for i in range(NJ)]
                for i in range(NJ):
                    nc.sync.dma_start(out=wq_sb[0][:, i, :], in_=wqg[0][:, i, :])
                    nc.sync.dma_start(out=hsT_sb[i][:, :], in_=hsT[ts(i, P), :])
                    if i == 0:
                        nc.sync.dma_start(out=xkvT_sb[:, :, :], in_=xkvTp)
                    elif i == 2:
                        nc.sync.dma_start(out=wq_sb[1][:, :, :], in_=wqg[1])
                    elif i == 4:
                        nc.sync.dma_start(out=wq_sb[2][:, :, :], in_=wqg[2])

                def q_pass(c, g):
                    j0, jw = JGS[g]
                    pss = [psq.tile([P, 512], F32, tag="qps", name="qps") for _ in range(jw)]
                    for i in range(NJ):
                        for jj in range(jw):
                            nc.tensor.matmul(
                                pss[jj][:, :],
                                wq_sb[g][:, i, ts(jj, P)],
                                hsT_sb[i][:, ts(c, 512)],
                                start=(i == 0), stop=(i == NJ - 1),
                            )
                    for jj in range(jw):
                        nc.scalar.copy(qT_sb[j0 + jj][:, ts(c, 512)], pss[jj][:, :])

                # kv chunk (proj, nn): 256 cols of my half of [k~|v]; proj 0
                # valid key rows 0:77 (ehs), proj 1 rows 96:128 (id embeds,
                # overwriting the proj-0 rows there; the 77:96 gap rows stay
                # garbage and are masked by the exp bias / zero probs).
                def kv_chunk(proj, nn):
                    ps = pskv.tile([P, 256], F32, tag="kvps", name="kvps",
                                   padded_shape=[P, 512])
                    for half in range(2):
                        w_t = wkvp.tile([P, 8, 256], R, tag="wkv", name="w_t")
                        nc.sync.dma_start(out=w_t[:, :, :], in_=wkvh[proj, nn, half])
                        for i8 in range(8):
                            i = half * 8 + i8
                            nc.tensor.matmul(
                                ps[:, :], xkvT_sb[:, i, :], w_t[:, i8, :],
                                start=(i == 0), stop=(i == NI - 1),
                            )
                    lo, hi = (0, P) if proj == 0 else (GAP1, P)
                    nc.scalar.copy(mykv[lo:hi, ts(nn, 256)], ps[lo:hi, :])

                kv_plan = [(p, nn) for nn in range(5) for p in range(2)]
                passes = [(c, g) for c in range(NCH) for g in range(len(JGS))]
                for pi, (c, g) in enumerate(passes):
                    q_pass(c, g)
                    if pi >= 1 and pi - 1 < len(kv_plan):
                        kv_chunk(*kv_plan[pi - 1])
                    if pi == len(kv_plan):
                        # swap the k~/v halves within each core pair via a
                        # full-group AllGather (sub-group comms hang on this
                        # runtime); core c reads back pieces 2*(c//2) (k~)
                        # and 2*(c//2)+1 (v).
                        kv_in = dram.tile([P, H], R, tag="kv_in")
                        kv_out = dram.tile([NCORES, P, H], R, tag="kv_out",
                                           addr_space="Shared")
                        nc.sync.dma_start(kv_in[:, :], mykv[:, :])
                        nc.gpsimd.collective_compute(
                            "AllGather",
                            mybir.AluOpType.bypass,
                            replica_groups=[list(range(NCORES))],
                            ins=[kv_in[:, :].opt()],
                            outs=[kv_out[:, :, :].opt()],
                        )
                        pid = nc.sync.partition_id()
                        kbase = (pid // 2) * 2
                        nc.sync.dma_start(out=kTMP[:, :], in_=kv_out[kbase])
                        nc.sync.dma_start(out=v_sb[:, :], in_=kv_out[kbase + 1])
                        for j in range(NJ):
                            nc.sync.dma_start(out=kT_sb[j][:, :],
                                              in_=kTMP[:, ts(j, P)], transpose=True)

            # O-projection weights + bias: allocated + DMA'd only now (the
            # SBUF they use was the hsT/kv-stream space during phase B+C)
            attnp2_cm = tc.tile_pool(name="attnp2", bufs=1, side="right")
            attnp2 = attnp2_cm.__enter__()
            boutb_sb = attnp2.tile([P, H], F32, tag="boutb")
            wout_sb = [attnp2.tile([P, H], R, tag=f"wout{i}", name=f"wout{i}") for i in range(NJ)]
            nc.sync.dma_start(out=boutb_sb[:, :], in_=boutb)
            for i in range(NJ):
                nc.sync.dma_start(out=wout_sb[i][:, :], in_=woutT[ts(i, P), :])

            # ---- phases A/E/O: attention with O-projection interleaved --------
            with (
                tc.tile_pool(name="pha", bufs=3) as pha,
                tc.tile_pool(name="psa", bufs=2, space="PSUM") as psa,
                tc.tile_pool(name="finp", bufs=2) as finp,
            ):
                astate = {}

                def attn_front(c, hp):
                    pts = []
                    for s in range(2):
                        rq = DH * s
                        ps_s = psa.tile([P, 512], F32, tag="sps", name="sps", bufs=4)
                        nc.tensor.matmul(
                            ps_s[:, :], kT_sb[hp][rq:rq + DH, :],
                            qT_sb[hp][rq:rq + DH, ts(c, 512)],
                            start=True, stop=True,
                        )
                        pts.append(ps_s)
                    probs = []
                    for s in range(2):
                        probsT = pha.tile([P, 512], R, tag="probsT", name="probsT", bufs=4)
                        nc.scalar.activation(
                            probsT[:, :], pts[s][:, :], EXP,
                            bias=bias_col[:, :], scale=SCALE,
                        )
                        probs.append(probsT)
                    astate[(c, hp)] = probs

                def attn_back(c, hp):
                    probs = astate.pop((c, hp))
                    # both heads share one PV psum tile (disjoint row halves)
                    # and one denominator tile (denom_h replicated over its
                    # own half by a ones stationary).
                    ps_o = psa.tile([P, 512], F32, tag="ops", name="ops")
                    ps_d = psa.tile([P, 512], F32, tag="dps", name="dps")
                    for s in range(2):
                        h = 2 * hp + s
                        rq = DH * s
                        nc.tensor.matmul(
                            ps_o[rq:rq + DH, :], v_sb[:, ts(h, DH)], probs[s][:, :],
                            start=True, stop=True,
                        )
                        nc.tensor.matmul(
                            ps_d[rq:rq + DH, :], ones_mat[:, 0:DH], probs[s][:, :],
                            start=True, stop=True,
                        )
                    bc_sb = pha.tile([P, 512], F32, tag="bc", name="bc_sb")
                    nc.vector.reciprocal_approx_fast(bc_sb[:, :], ps_d[:, :])
                    nc.vector.tensor_mul(
                        attnT_sb[hp][:, ts(c, 512)], ps_o[:, :], bc_sb[:, :]
                    )

                def o_tile(t):
                    for m0, mw in MCHUNKS:
                        # shares the PV ("ops") bank rotation: attention and
                        # O-projection together stay within the 8 PSUM banks
                        psf = psa.tile([P, mw], F32, tag="ops", name="psf",
                                       padded_shape=[P, 512])
                        for i in range(NJ):
                            nc.tensor.matmul(
                                psf[:, :], attnT_sb[i][:, ts(t, P)],
                                wout_sb[i][:, m0:m0 + mw],
                                start=(i == 0), stop=(i == NJ - 1),
                            )
                        fin = finp.tile([P, 512], F32, tag="fin", name="fin",
                                        bufs=3)
                        nc.vector.tensor_add(
                            fin[:, 0:mw], psf[:, :], boutb_sb[:, m0:m0 + mw]
                        )
                        nc.sync.dma_start(out=out[ts(t, P), m0:m0 + mw],
                                          in_=fin[:, 0:mw])

                # chunk c0: attention only (1-deep front/back pipeline)
                prev = None
                for hp in range(NJ):
                    attn_front(0, hp)
                    if prev is not None:
                        attn_back(*prev)
                    prev = (0, hp)
                # chunks c1..c3: interleave 1 O-tile of chunk c-1 after every
                # 2-3 pairs so ACT's exp stream hides under O matmuls.
                for c in range(1, NCH):
                    osl = [4 * (c - 1), None, 4 * (c - 1) + 1, None,
                           4 * (c - 1) + 2, None, None, 4 * (c - 1) + 3, None, None]
                    for hp in range(NJ):
                        attn_front(c, hp)
                        if prev is not None:
                            attn_back(*prev)
                        prev = (c, hp)
                        if osl[hp] is not None:
                            o_tile(osl[hp])
                attn_back(*prev)
                # last chunk's O tiles
                for t in range(4 * (NCH - 1), NT):
                    o_tile(t)

            qTp_cm.__exit__(None, None, None)
            attnp2_cm.__exit__(None, None, None)
            attnp_cm.__exit__(None, None, None)

    nc.compile()
    _NC_CACHE["nc"] = nc
    return nc


def prep_core_inputs(hidden_states, encoder_hidden_states, id_embedding,
                     Wq, Wk, Wv, Wid_k, Wid_v, Wout, bout):
    """Host-side sharding / layout prep.  Returns list of 8 in_maps."""
    f = np.float32
    h16 = np.float16
    hidden_states = np.asarray(hidden_states, f)
    encoder_hidden_states = np.asarray(encoder_hidden_states, f)
    id_embedding = np.asarray(id_embedding, f)
    Wq = np.asarray(Wq, f)
    Wout = np.asarray(Wout, f)
    Wk, Wv = np.asarray(Wk, f), np.asarray(Wv, f)
    Wid_k, Wid_v = np.asarray(Wid_k, f), np.asarray(Wid_v, f)
    boutb = np.ascontiguousarray(np.broadcast_to(np.asarray(bout, f), (P, H)))

    # wq per j-group: [128, 10(i), jw*128], element [p, i, jj*128+f] =
    # Wq[i*128+p, (j0+jj)*128+f]  (big contiguous per-partition DMA rows)
    wq4 = Wq.reshape(NJ, P, NJ, P)                       # [i, p, j, f]
    wqg = []
    for (j0, jw) in JGS:
        g = wq4[:, :, j0:j0 + jw, :]                     # [i, p, jw, f]
        wqg.append(np.ascontiguousarray(
            g.transpose(1, 0, 2, 3).reshape(P, NJ, jw * P).astype(h16)))

    # kv weight halves: parity 0 cores stream the k~ columns, parity 1 the v
    # columns.  [proj, nn, half, p, i8, f] = W[(half*8+i8)*128+p,
    # parity*1280 + nn*256 + f]
    wkv = np.concatenate([Wk, Wv], axis=1)               # [C, 2H]
    widkv = np.concatenate([Wid_k, Wid_v], axis=1)
    wkvh_par = []
    for parity in range(2):
        wh = np.empty((2, 5, 2, P, 8, 256), h16)
        for proj, W in enumerate([wkv, widkv]):
            Wh = W[:, parity * H:(parity + 1) * H]       # [C, H]
            w6 = Wh.reshape(2, 8, P, 5, 256)             # [half, i8, p, nn, f]
            wh[proj] = w6.transpose(3, 0, 2, 1, 4).astype(h16)
        wkvh_par.append(wh)

    wout16 = np.ascontiguousarray(Wout.astype(h16))
    in_maps = []
    for core in range(NCORES):
        b, hf = divmod(core, 2)
        hsT = np.ascontiguousarray(hidden_states[b, hf * SC:(hf + 1) * SC, :].T.astype(h16))
        xkvT = np.zeros((C, LP), h16)                    # [C, 128]
        xkvT[:, :TE] = encoder_hidden_states[b].T
        xkvT[:, GAP1:] = id_embedding[b % 2].T
        # pack partition-major: [p, i, kk] = xkvT[i*128+p, kk]
        xkvTp = np.ascontiguousarray(xkvT.reshape(NI, P, LP).transpose(1, 0, 2))
        m = {
            "hsT": hsT, "xkvTp": xkvTp, "wkvh": wkvh_par[core % 2],
            "woutT": wout16, "boutb": boutb,
        }
        for g in range(len(JGS)):
            m[f"wqg{g}"] = wqg[g]
        in_maps.append(m)
    return in_maps


def kernel(hidden_states, encoder_hidden_states, id_embedding,
           Wq, Wk, Wv, Wid_k, Wid_v, Wout, bout, _trace=False):
    _ensure_axon_hooks()
    from concourse.bass_utils import run_bass_kernel_spmd

    nc = build_nc()
    in_maps = prep_core_inputs(hidden_states, encoder_hidden_states, id_embedding,
                               Wq, Wk, Wv, Wid_k, Wid_v, Wout, bout)
    kwargs = {}
    if _trace:
        import concourse.bass_utils as bu
        bu.upload_artifacts = lambda tmpdir: f"local://{tmpdir}"
        kwargs["trace"] = True
    res = run_bass_kernel_spmd(nc, in_maps, core_ids=list(range(NCORES)), **kwargs)

    outp = np.empty((B, S, H), np.float32)
    for core in range(NCORES):
        b, hf = divmod(core, 2)
        outp[b, hf * SC:(hf + 1) * SC, :] = res.results[core]["out"]
    if _trace:
        kernel.last_exec_time_ns = res.exec_time_ns
        kernel.last_results = res
    return outp


# revision 22
# speedup vs baseline: 1.3915x; 1.3915x over previous
"""Trainium2 Bass kernel for the branched cross-attention processor.

Problem (full shapes):
  hidden_states [4, 4096, 1280], encoder_hidden_states [4, 77, 2048],
  id_embedding [2, 32, 2048], Wq/Wout [1280,1280], Wk/Wv/Wid_k/Wid_v
  [2048,1280], bout [1280].  20 heads, dh=64.  Output [4, 4096, 1280].

Sharding: data-parallel over (batch, seq-half): core c handles batch c//2,
query rows (c%2)*2048 : (c%2+1)*2048.  K/V (109 keys) are computed
per-core for its batch.  No collectives.

v2 schedule (PE-roofline oriented, ~255us of fp16 PE work per core):
  B. Q projection, i-outer: for c-chunk, for j-group: accumulate over all
     10 input tiles into <=4 PSUM banks.  Starts as soon as hsT tile 0 +
     the j-group's weights land (weights packed for big-burst DMA).
  C. KV projection (both [Wk|Wv] and [Wid_k|Wid_v] passes); the 21MB
     weight stream overlaps phase B's 85us of PE work.  kT via DMA
     transpose; v stays key-major.
  A. Attention chunk c0 (pairs of heads; scores -> exp -> PV + ones-matmul
     denominator -> reciprocal -> normalize), then chunks c1..c3 with the
     O-projection tiles of chunk c-1 interleaved 2 pairs : 1 tile so the
     ACT exp stream hides under O matmuls and the PE never stalls.
  O. Remaining O tiles of chunk c3 + bias + output DMA.

Engines: PE matmuls; ACT exp + PSUM->SBUF fp16 copies; DVE reciprocal +
bias add; Pool (gpsimd) the normalize multiply.
"""

import os
import sys
import types

import numpy as np

# ---------------------------------------------------------------------------
# problem constants (hardcoded; kernel.py must be self-contained)
# ---------------------------------------------------------------------------
B = 4
S = 4096
H = 1280
C = 2048
TE = 77          # encoder tokens
TI = 32          # id tokens
HEADS = 20
DH = 64          # head dim
P = 128
L = 109          # TE + TI
LP = 128         # padded key count: [0:77]=ehs, [77:96]=gap, [96:128]=id
GAP0, GAP1 = TE, P - TI   # 77, 96
SC = 2048        # seq rows per core
NJ = H // P      # 10
NI = C // P      # 16
NCH = SC // 512  # 4 sq-chunks of 512
NT = SC // P     # 16 sq-tiles of 128
SCALE = 1.0 / 8.0
NCORES = 8
MCHUNKS = [(0, 512), (512, 512), (1024, 256)]
JGS = [(0, 4), (4, 3), (7, 3)]   # j-groups for the i-outer Q passes

_NC_CACHE = {}


def _ensure_axon_hooks():
    """The image's antenv lacks axon_hooks; synthesize it so NTFF profiling
    (trace=True) works when test.py asks for it.  Harmless if unused."""
    if "antenv.axon_hooks" in sys.modules:
        return
    try:
        import antenv
        from trn_agent_boot.trn_boot import _ntff_profile_via_ctypes

        hook = _ntff_profile_via_ctypes("/opt/axon/libaxon_pjrt.so")
        m = types.ModuleType("antenv.axon_hooks")
        m.get_axon_ntff_profile_hook = lambda: hook
        m.set_axon_ntff_profile_hook = lambda h: None
        sys.modules["antenv.axon_hooks"] = m
        antenv.axon_hooks = m
    except Exception:
        pass


def build_nc():
    """Build + compile the per-core Bass program (SPMD: same NEFF, 8 cores)."""
    if "nc" in _NC_CACHE:
        return _NC_CACHE["nc"]

    import concourse.bass as bass
    import concourse.tile as tile
    from concourse import bacc, mybir
    from concourse.bass import ts

    F32 = mybir.dt.float32
    R = mybir.dt.float16      # matmul operand dtype (1 cyc/row)
    EXP = mybir.ActivationFunctionType.Exp

    nc = bacc.Bacc("TRN2", target_bir_lowering=False, debug=False, num_devices=NCORES)

    hsT = nc.dram_tensor("hsT", [H, SC], R, kind="ExternalInput").ap()
    # xkvTp: [128, 16, 128]  (partition-major packed transposed kv inputs)
    xkvTp = nc.dram_tensor("xkvTp", [P, NI, LP], R, kind="ExternalInput").ap()
    # wq packed per j-group: [128, 10, jw*128]
    wqg = [nc.dram_tensor(f"wqg{g}", [P, NJ, jw * P], R, kind="ExternalInput").ap()
           for g, (j0, jw) in enumerate(JGS)]
    # kv weights: [proj, n, half, 128, 8, 512]
    wkvs = nc.dram_tensor("wkvs", [2, 5, 2, P, 8, 512], R, kind="ExternalInput").ap()
    woutT = nc.dram_tensor("woutT", [H, H], R, kind="ExternalInput").ap()
    boutb = nc.dram_tensor("boutb", [P, H], F32, kind="ExternalInput").ap()
    out = nc.dram_tensor("out", [SC, H], F32, kind="ExternalOutput").ap()

    with tile.TileContext(nc) as tc:
        with tc.tile_pool(name="pers", bufs=1) as pers:
            # ---- persistent constants / small arrays --------------------------
            ones_mat = pers.tile([P, P], R, tag="ones_mat")
            nc.vector.memset(ones_mat[:, :], 1.0)
            bias_col = pers.tile([P, 1], F32, tag="bias_col")
            # engine ops need 32-aligned start partitions: write the gap
            # as [64:96] then restore [64:77]; later writes overwrite cleanly.
            nc.vector.memset(bias_col[:, :], 0.0)
            nc.vector.memset(bias_col[64:GAP1, :], -1e30)
            nc.vector.memset(bias_col[64:GAP0, :], 0.0)
            kT_sb = [pers.tile([P, LP], R, tag=f"kT{j}", name=f"kT{j}") for j in range(NJ)]
            v_sb = pers.tile([LP, HEADS * DH], R, tag="v")
            xkvT_sb = pers.tile([P, NI, LP], R, tag="xkvT")
            wq_sb = [pers.tile([P, NJ, jw * P], R, tag=f"wqg{g}", name=f"wqg{g}")
                     for g, (j0, jw) in enumerate(JGS)]

            # right-side stack: tensors that live until the end
            attnp_cm = tc.tile_pool(name="attnp", bufs=1, side="right")
            attnp = attnp_cm.__enter__()
            attnT_sb = [attnp.tile([P, SC], R, tag=f"attnT{d}", name=f"attnT{d}") for d in range(NJ)]

            qTp_cm = tc.tile_pool(name="qTp", bufs=1)
            qTp = qTp_cm.__enter__()
            qT_sb = [qTp.tile([P, SC], R, tag=f"qT{j}", name=f"qT{j}") for j in range(NJ)]

            kTMP = pers.tile([P, H], R, tag="kTMP")

            # ---- phase B+C: q projection (i-outer passes) with the kv
            # projection chunks interleaved between passes so the 21MB kv
            # weight stream spreads over the whole ~120us window ------------
            with (
                tc.tile_pool(name="wkvp", bufs=3) as wkvp,
                tc.tile_pool(name="phq", bufs=1) as phq,
                tc.tile_pool(name="psq", bufs=6, space="PSUM") as psq,
                tc.tile_pool(name="pskv", bufs=2, space="PSUM") as pskv,
            ):
                # DMA order = priority order: xkv (kv chunk 0), then the
                # first j-group's weights interleaved with the hsT stream
                # (pass (c0, g0) consumes hsT tile-by-tile), then the rest.
                # pass (c0, g0) only reads hsT cols 0:512, so stream those
                # narrow slices first and backfill the bulk afterwards.
                hsT_sb = [phq.tile([P, SC], R, tag=f"hsT{i}", name=f"hsT{i}") for i in range(NJ)]
                for i in range(NJ):
                    nc.sync.dma_start(out=wq_sb[0][:, i, :], in_=wqg[0][:, i, :])
                    nc.sync.dma_start(out=hsT_sb[i][:, 0:512], in_=hsT[ts(i, P), 0:512])
                    if i == 0:
                        nc.sync.dma_start(out=xkvT_sb[:, :, :], in_=xkvTp)
                nc.sync.dma_start(out=wq_sb[1][:, :, :], in_=wqg[1])
                for i in range(NJ):
                    nc.sync.dma_start(out=hsT_sb[i][:, 512:SC], in_=hsT[ts(i, P), 512:SC])
                    if i == 4:
                        nc.sync.dma_start(out=wq_sb[2][:, :, :], in_=wqg[2])

                def q_pass(c, g):
                    j0, jw = JGS[g]
                    pss = [psq.tile([P, 512], F32, tag="qps", name="qps") for _ in range(jw)]
                    for i in range(NJ):
                        for jj in range(jw):
                            nc.tensor.matmul(
                                pss[jj][:, :],
                                wq_sb[g][:, i, ts(jj, P)],
                                hsT_sb[i][:, ts(c, 512)],
                                start=(i == 0), stop=(i == NJ - 1),
                            )
                    for jj in range(jw):
                        # split PSUM->SBUF copies across ACT and DVE so the
                        # next pass's PSUM-slot reuse isn't copy-latency bound
                        if jj % 2 == 0:
                            nc.scalar.copy(qT_sb[j0 + jj][:, ts(c, 512)], pss[jj][:, :])
                        else:
                            nc.vector.tensor_copy(qT_sb[j0 + jj][:, ts(c, 512)], pss[jj][:, :])

                # kv chunk (proj, n): 512 output cols of [Wk|Wv] (proj 0,
                # valid key rows 0:77) or [Wid_k|Wid_v] (proj 1, rows
                # 96:128).  k cols first, so kT transposes start early.
                def kv_chunk(proj, n):
                    ps = pskv.tile([P, 512], F32, tag="kvps", name="kvps")
                    for half in range(2):
                        w_t = wkvp.tile([P, 8, 512], R, tag="wkv", name="w_t")
                        nc.sync.dma_start(out=w_t[:, :, :], in_=wkvs[proj, n, half])
                        for i8 in range(8):
                            i = half * 8 + i8
                            nc.tensor.matmul(
                                ps[:, :], xkvT_sb[:, i, :], w_t[:, i8, :],
                                start=(i == 0), stop=(i == NI - 1),
                            )
                    lo, hi = (0, P) if proj == 0 else (GAP1, P)
                    if n < 2:
                        nc.scalar.copy(kTMP[lo:hi, ts(n, 512)], ps[lo:hi, :])
                    elif n == 2:
                        nc.scalar.copy(kTMP[lo:hi, 1024:1280], ps[lo:hi, 0:256])
                        nc.scalar.copy(v_sb[lo:hi, 0:256], ps[lo:hi, 256:512])
                    else:
                        v0 = 512 * n - 1280
                        nc.scalar.copy(v_sb[lo:hi, v0:v0 + 512], ps[lo:hi, :])
                    if (proj, n) == (1, 2):
                        # all k columns final -> transpose k~ into kT
                        for j in range(NJ):
                            nc.sync.dma_start(out=kT_sb[j][:, :],
                                              in_=kTMP[:, ts(j, P)], transpose=True)

                kv_plan = [(0, 0), (0, 1), (0, 2), (1, 0), (1, 1), (1, 2),
                           (0, 3), (0, 4), (1, 3), (1, 4)]
                passes = [(c, g) for c in range(NCH) for g in range(len(JGS))]
                for pi, (c, g) in enumerate(passes):
                    q_pass(c, g)
                    if pi >= 2 and pi - 2 < len(kv_plan):
                        kv_chunk(*kv_plan[pi - 2])

            # O-projection weights + bias: allocated + DMA'd only now (the
            # SBUF they use was the hsT/kv-stream space during phase B+C)
            attnp2_cm = tc.tile_pool(name="attnp2", bufs=1, side="right")
            attnp2 = attnp2_cm.__enter__()
            boutb_sb = attnp2.tile([P, H], F32, tag="boutb")
            wout_sb = [attnp2.tile([P, H], R, tag=f"wout{i}", name=f"wout{i}") for i in range(NJ)]
            nc.sync.dma_start(out=boutb_sb[:, :], in_=boutb)
            for i in range(NJ):
                nc.sync.dma_start(out=wout_sb[i][:, :], in_=woutT[ts(i, P), :])

            # ---- phases A/E/O: attention with O-projection interleaved --------
            with (
                tc.tile_pool(name="pha", bufs=3) as pha,
                tc.tile_pool(name="psa", bufs=2, space="PSUM") as psa,
                tc.tile_pool(name="finp", bufs=2) as finp,
            ):
                astate = {}

                def attn_front(c, hp):
                    pts = []
                    for s in range(2):
                        rq = DH * s
                        ps_s = psa.tile([P, 512], F32, tag="sps", name="sps", bufs=4)
                        nc.tensor.matmul(
                            ps_s[:, :], kT_sb[hp][rq:rq + DH, :],
                            qT_sb[hp][rq:rq + DH, ts(c, 512)],
                            start=True, stop=True,
                        )
                        pts.append(ps_s)
                    probs = []
                    for s in range(2):
                        probsT = pha.tile([P, 512], R, tag="probsT", name="probsT", bufs=4)
                        nc.scalar.activation(
                            probsT[:, :], pts[s][:, :], EXP,
                            bias=bias_col[:, :], scale=SCALE,
                        )
                        probs.append(probsT)
                    astate[(c, hp)] = probs

                def attn_back(c, hp):
                    probs = astate.pop((c, hp))
                    # both heads share one PV psum tile (disjoint row halves)
                    # and one denominator tile (denom_h replicated over its
                    # own half by a ones stationary).
                    ps_o = psa.tile([P, 512], F32, tag="ops", name="ops")
                    ps_d = psa.tile([P, 512], F32, tag="dps", name="dps")
                    for s in range(2):
                        h = 2 * hp + s
                        rq = DH * s
                        nc.tensor.matmul(
                            ps_o[rq:rq + DH, :], v_sb[:, ts(h, DH)], probs[s][:, :],
                            start=True, stop=True,
                        )
                        nc.tensor.matmul(
                            ps_d[rq:rq + DH, :], ones_mat[:, 0:DH], probs[s][:, :],
                            start=True, stop=True,
                        )
                    bc_sb = pha.tile([P, 512], F32, tag="bc", name="bc_sb")
                    nc.vector.reciprocal_approx_fast(bc_sb[:, :], ps_d[:, :])
                    nc.vector.tensor_mul(
                        attnT_sb[hp][:, ts(c, 512)], ps_o[:, :], bc_sb[:, :]
                    )

                def o_tile(t):
                    for m0, mw in MCHUNKS:
                        # shares the PV ("ops") bank rotation: attention and
                        # O-projection together stay within the 8 PSUM banks
                        psf = psa.tile([P, mw], F32, tag="ops", name="psf",
                                       padded_shape=[P, 512])
                        for i in range(NJ):
                            nc.tensor.matmul(
                                psf[:, :], attnT_sb[i][:, ts(t, P)],
                                wout_sb[i][:, m0:m0 + mw],
                                start=(i == 0), stop=(i == NJ - 1),
                            )
                        fin = finp.tile([P, 512], F32, tag="fin", name="fin",
                                        bufs=3)
                        nc.vector.tensor_add(
                            fin[:, 0:mw], psf[:, :], boutb_sb[:, m0:m0 + mw]
                        )
                        nc.sync.dma_start(out=out[ts(t, P), m0:m0 + mw],
                                          in_=fin[:, 0:mw])

                # chunk c0: attention only (1-deep front/back pipeline)
                prev = None
                for hp in range(NJ):
                    attn_front(0, hp)
                    if prev is not None:
                        attn_back(*prev)
                    prev = (0, hp)
                # chunks c1..c3: interleave 1 O-tile of chunk c-1 after every
                # 2-3 pairs so ACT's exp stream hides under O matmuls.
                for c in range(1, NCH):
                    osl = [4 * (c - 1), None, 4 * (c - 1) + 1, None,
                           4 * (c - 1) + 2, None, None, 4 * (c - 1) + 3, None, None]
                    for hp in range(NJ):
                        attn_front(c, hp)
                        if prev is not None:
                            attn_back(*prev)
                        prev = (c, hp)
                        if osl[hp] is not None:
                            o_tile(osl[hp])
                attn_back(*prev)
                # last chunk's O tiles
                for t in range(4 * (NCH - 1), NT):
                    o_tile(t)

            qTp_cm.__exit__(None, None, None)
            attnp2_cm.__exit__(None, None, None)
            attnp_cm.__exit__(None, None, None)

    nc.compile()
    _NC_CACHE["nc"] = nc
    return nc


def prep_core_inputs(hidden_states, encoder_hidden_states, id_embedding,
                     Wq, Wk, Wv, Wid_k, Wid_v, Wout, bout):
    """Host-side sharding / layout prep.  Returns list of 8 in_maps."""
    f = np.float32
    h16 = np.float16
    hidden_states = np.asarray(hidden_states, f)
    encoder_hidden_states = np.asarray(encoder_hidden_states, f)
    id_embedding = np.asarray(id_embedding, f)
    Wq = np.asarray(Wq, f)
    Wout = np.asarray(Wout, f)
    Wk, Wv = np.asarray(Wk, f), np.asarray(Wv, f)
    Wid_k, Wid_v = np.asarray(Wid_k, f), np.asarray(Wid_v, f)
    boutb = np.ascontiguousarray(np.broadcast_to(np.asarray(bout, f), (P, H)))

    # wq per j-group: [128, 10(i), jw*128], element [p, i, jj*128+f] =
    # Wq[i*128+p, (j0+jj)*128+f]  (big contiguous per-partition DMA rows)
    wq4 = Wq.reshape(NJ, P, NJ, P)                       # [i, p, j, f]
    wqg = []
    for (j0, jw) in JGS:
        g = wq4[:, :, j0:j0 + jw, :]                     # [i, p, jw, f]
        wqg.append(np.ascontiguousarray(
            g.transpose(1, 0, 2, 3).reshape(P, NJ, jw * P).astype(h16)))

    # kv weights: [proj, n, half, p, i8, f] with element =
    # W[(half*8+i8)*128+p, n*512+f]
    wkv = np.concatenate([Wk, Wv], axis=1)               # [C, 2H]
    widkv = np.concatenate([Wid_k, Wid_v], axis=1)
    wkvs = np.empty((2, 5, 2, P, 8, 512), h16)
    for proj, W in enumerate([wkv, widkv]):
        w6 = W.reshape(2, 8, P, 5, 512)                  # [half, i8, p, n, f]
        wkvs[proj] = w6.transpose(3, 0, 2, 1, 4).astype(h16)  # [n, half, p, i8, f]

    wout16 = np.ascontiguousarray(Wout.astype(h16))
    in_maps = []
    for core in range(NCORES):
        b, hf = divmod(core, 2)
        hsT = np.ascontiguousarray(hidden_states[b, hf * SC:(hf + 1) * SC, :].T.astype(h16))
        xkvT = np.zeros((C, LP), h16)                    # [C, 128]
        xkvT[:, :TE] = encoder_hidden_states[b].T
        xkvT[:, GAP1:] = id_embedding[b % 2].T
        # pack partition-major: [p, i, kk] = xkvT[i*128+p, kk]
        xkvTp = np.ascontiguousarray(xkvT.reshape(NI, P, LP).transpose(1, 0, 2))
        m = {
            "hsT": hsT, "xkvTp": xkvTp, "wkvs": wkvs,
            "woutT": wout16, "boutb": boutb,
        }
        for g in range(len(JGS)):
            m[f"wqg{g}"] = wqg[g]
        in_maps.append(m)
    return in_maps


def kernel(hidden_states, encoder_hidden_states, id_embedding,
           Wq, Wk, Wv, Wid_k, Wid_v, Wout, bout, _trace=False):
    _ensure_axon_hooks()
    from concourse.bass_utils import run_bass_kernel_spmd

    nc = build_nc()
    in_maps = prep_core_inputs(hidden_states, encoder_hidden_states, id_embedding,
                               Wq, Wk, Wv, Wid_k, Wid_v, Wout, bout)
    kwargs = {}
    if _trace:
        import concourse.bass_utils as bu
        bu.upload_artifacts = lambda tmpdir: f"local://{tmpdir}"
        kwargs["trace"] = True
    res = run_bass_kernel_spmd(nc, in_maps, core_ids=list(range(NCORES)), **kwargs)

    outp = np.empty((B, S, H), np.float32)
    for core in range(NCORES):
        b, hf = divmod(core, 2)
        outp[b, hf * SC:(hf + 1) * SC, :] = res.results[core]["out"]
    if _trace:
        kernel.last_exec_time_ns = res.exec_time_ns
        kernel.last_results = res
    return outp
